# revision 15
# baseline (speedup 1.0000x reference)
"""Trainium2 Bass kernel: per-sample dynamic conv (KernelAggregation).

Problem: out[b] = conv2d(x[b], sum_n att[b,n]*W[n], pad=1) + (att @ bias)[b]
  x: (16, 256, 56, 56) f32, att: (16, 8), W: (8, 256, 256, 3, 3), bias: (8, 256)

Sharding: data-parallel over batch, 2 samples per core across 8 cores.

The axon tunnel moves ~40-80 MB/s and the host has a single CPU, so wall
time is wire bytes + host byte-shuffling; device compute (~0.4 ms/core) is
free. Design:
  * x ships as bf16 (25.7 MB, not 51.4).
  * The weight bank ships ONCE, sharded (9.4 MB bf16, one kernel per
    core) in its native layout; a small stage-1 jax jit all-gathers and
    transposes it on device into matmul layout. No host transpose, no
    host mixing sgemm, no 151 MB replication.
  * The Bass kernel mixes per-sample conv weights on DVE via
    scalar_tensor_tensor FMA (acc = att[s,n]*bank[n] + acc), then runs
    the conv as 9 shifted bf16 matmuls accumulated in PSUM.
  * y returns as int8 (12.9 MB) with device-computed per-(sample,channel)
    abs-max scales (exact dynamic quantization: DVE abs-max reduction ->
    reciprocal -> ACT rescale+cast); the host dequantizes. Measured
    rel-err vs the fp32 reference stays ~1e-2, well inside 2e-2.
  * Everything dispatches async; the only blocking point is the final y
    fetch. The shard_map jit is built once at module scope; warm calls
    pay only transfers + one dispatch chain.
  * The donated-zero output upload of run_bass_kernel_spmd's axon path is
    replaced by persistent on-device dummy operands (the kernel writes
    every output element, so zero-init is unnecessary).
"""

import numpy as np
from contextlib import ExitStack

B, DIM, H, W = 16, 256, 56, 56
NK, KS = 8, 3
NCORES = 8
SPC = B // NCORES          # samples per core
S = W + 2                  # padded row stride (58)
NPAD = S * S               # 3364
XP_LEN = NPAD + 4          # slack so shifted reads stay in-bounds (even)
ROWS_PER_T = 8
NT = H // ROWS_PER_T       # 7 spatial tiles
NTILE = ROWS_PER_T * S     # 464 (matmul moving dim)
CI_CH = DIM // 128         # 2
CO_CH = DIM // 128         # 2
KK = KS * KS               # 9
NG = SPC * CO_CH           # quantization groups per core (4)

NPS = 4     # PSUM tiles
NOUT = 4    # output staging buffers


def _imports():
    try:
        import concourse.bass as bass  # noqa: F401
    except ImportError:
        import sys
        for p in ("/opt/trn_rl_repo",):
            if p not in sys.path:
                sys.path.insert(0, p)
    import concourse.bass as bass
    import concourse.tile as tile
    from concourse import mybir
    return bass, tile, mybir


def build_bass_raw():
    bass, tile, mybir = _imports()
    dt = mybir.dt
    nc = bass.Bass()

    x = nc.dram_tensor("x", [SPC, DIM, H, W], dt.bfloat16, kind="ExternalInput")
    # Device-gathered+transposed bank from the stage-1 jit: [n, ci, kk*co].
    gbank = nc.dram_tensor("gbank", [NK, CI_CH, 128, KK * DIM], dt.bfloat16,
                           kind="ExternalInput")
    attb = nc.dram_tensor("attb", [128, SPC * NK], dt.float32,
                          kind="ExternalInput")
    bmixT = nc.dram_tensor("bmixT", [128, CO_CH * SPC], dt.float32,
                           kind="ExternalInput")
    y = nc.dram_tensor("y", [SPC, DIM, H, W], dt.int8, kind="ExternalOutput")
    yscale = nc.dram_tensor("yscale", [128, NG], dt.float32,
                            kind="ExternalOutput")

    ctx = ExitStack()
    with ctx:
        sbm = lambda shape, name: ctx.enter_context(
            nc.sbuf_tensor(name, shape, dt.bfloat16))
        sbf = lambda shape, name: ctx.enter_context(
            nc.sbuf_tensor(name, shape, dt.float32))
        att_sb = sbf([128, SPC * NK], "att_sb")
        bmix_sb = sbf([128, CO_CH * SPC], "bmix_sb")
        xp = [[sbm([128, XP_LEN], f"xp{s}_{c}") for c in range(CI_CH)]
              for s in range(SPC)]
        bank = [[sbm([128, KK * DIM], f"bk{n}_{c}") for c in range(CI_CH)]
                for n in range(NK)]
        acc = [[sbf([128, KK * DIM], f"acc{s}_{c}") for c in range(CI_CH)]
               for s in range(SPC)]
        wm = [[sbm([128, KK * DIM], f"wm{s}_{c}") for c in range(CI_CH)]
              for s in range(SPC)]
        # Pre-quant y tiles, bf16, reusing the (consumed) acc region:
        # group g=(s,co) holds NT*NTILE=3248 of acc's 4608-bf16 capacity.
        obuf = [acc[g // CO_CH][g % CO_CH][:].bitcast(dt.bfloat16)
                for g in range(NG)]
        oti = [ctx.enter_context(nc.sbuf_tensor(f"oti{i}", [128, NTILE],
                                                dt.int8))
               for i in range(NOUT)]
        amax_sb = sbf([128, NG], "amax_sb")
        qs_sb = sbf([128, NG], "qs_sb")
        inv_sb = sbf([128, NG], "inv_sb")
        red0 = sbm([128, NT * NTILE], "red0")
        red1 = sbm([128, (NT * NTILE + 1) // 2], "red1")
        psum = [ctx.enter_context(nc.psum_tensor(f"ps{i}", [128, NTILE],
                                                 dt.float32))
                for i in range(NPS)]

        sem = lambda name: ctx.enter_context(nc.semaphore(name))
        sem_small = sem("sem_small")   # att+bmix DMA done (2x16)
        sem_ms = sem("sem_ms")         # DVE memsets done (1 each, 4)
        sem_x = sem("sem_x")           # x interior DMA done (4x16)
        sem_bank = sem("sem_bank")     # bank DMA (n,c) done at 16*(2n+c+1)
        sem_wm = sem("sem_wm")         # mixed weights (s,c) ready (4)
        sem_mm = sem("sem_mm")         # PE per-out-tile group done (28)
        sem_a1 = sem("sem_a1")         # ACT psum->obuf tiles (28)
        sem_qs = sem("sem_qs")         # DVE per-group qscale ready (4)
        sem_tree = sem("sem_tree")     # DVE reduce-tree level serializer
        sem_a2 = sem("sem_a2")         # ACT quantized tiles (28)
        sem_outdma = sem("sem_outdma")  # out DMA done (16 each, 28+1)

        Ident = mybir.ActivationFunctionType.Identity
        Copy = mybir.ActivationFunctionType.Copy
        Alu = mybir.AluOpType

        groups = [(s, co) for s in range(SPC) for co in range(CO_CH)]
        tiles = [(s, co, t) for (s, co) in groups for t in range(NT)]

        def obuf_tile(g, t):
            return obuf[g][:, t * NTILE:(t + 1) * NTILE]

        # ---------------- GPSIMD: all input DMAs
        nc.gpsimd.dma_start(att_sb[:], attb[:, :]).then_inc(sem_small, 16)
        nc.gpsimd.dma_start(bmix_sb[:], bmixT[:, :]).then_inc(sem_small, 16)
        for n in range(NK):
            for c in range(CI_CH):
                nc.gpsimd.dma_start(bank[n][c][:],
                                    gbank[n, c, :, :]).then_inc(sem_bank, 16)
        for i, (s, c) in enumerate([(s, c) for s in range(SPC)
                                    for c in range(CI_CH)]):
            nc.gpsimd.wait_ge(sem_ms, i + 1)
            interior = xp[s][c][:, :NPAD].rearrange(
                "p (r u) -> p r u", u=S)[:, 1:1 + H, 1:1 + W]
            nc.gpsimd.dma_start(
                interior, x[s, c * 128:(c + 1) * 128, :, :]).then_inc(sem_x, 16)

        # ---------------- DVE: memsets; weight mixing; y quant scales
        for s in range(SPC):
            for c in range(CI_CH):
                nc.vector.memset(xp[s][c][:].bitcast(dt.float32),
                                 0.0).then_inc(sem_ms, 1)
        nc.vector.wait_ge(sem_small, 16)   # att_sb loaded
        for n in range(NK):
            for c in range(CI_CH):
                nc.vector.wait_ge(sem_bank, 16 * (2 * n + c + 1))
                for s in range(SPC):
                    a = att_sb[:, s * NK + n: s * NK + n + 1]
                    if n == 0:
                        nc.vector.tensor_scalar_mul(
                            acc[s][c][:], bank[n][c][:], a)
                    else:
                        nc.vector.scalar_tensor_tensor(
                            acc[s][c][:], bank[n][c][:], a, acc[s][c][:],
                            Alu.mult, Alu.add)
        for s in range(SPC):
            for c in range(CI_CH):
                nc.vector.tensor_copy(wm[s][c][:],
                                      acc[s][c][:]).then_inc(sem_wm, 1)
        # Per-group abs-max over the 7 bf16 tiles: |x| = (x*-1) max x, then
        # a pairwise halving max tree (odd sizes overlap the middle column,
        # harmless for max). Dependent back-to-back DVE ops with partial
        # overlap misexecute on this toolchain, so every level is
        # serialized with a self-semaphore. tensor_reduce/abs_max
        # miscompile here, hence only mult/max STT ops.
        jt = 0
        for g in range(NG):
            ga = amax_sb[:, g:g + 1]
            nc.vector.wait_ge(sem_a1, (g + 1) * NT)
            n = NT * NTILE
            nc.vector.scalar_tensor_tensor(
                red0[:, :n], obuf[g][:, :n], -1.0, obuf[g][:, :n],
                Alu.mult, Alu.max).then_inc(sem_tree, 1)
            jt += 1
            cur, other = red0, red1
            while n > 1:
                h = (n + 1) // 2
                nc.vector.wait_ge(sem_tree, jt)
                nc.vector.scalar_tensor_tensor(
                    other[:, :h], cur[:, :h], 0.0, cur[:, n - h:n],
                    Alu.max, Alu.max).then_inc(sem_tree, 1)
                jt += 1
                cur, other = other, cur
                n = h
            nc.vector.wait_ge(sem_tree, jt)
            nc.vector.tensor_scalar_max(ga, cur[:, 0:1],
                                        1e-30).then_inc(sem_tree, 1)
            jt += 1
            nc.vector.wait_ge(sem_tree, jt)
            nc.vector.reciprocal(inv_sb[:, g:g + 1], ga).then_inc(sem_tree, 1)
            jt += 1
            nc.vector.wait_ge(sem_tree, jt)
            nc.vector.tensor_scalar_mul(qs_sb[:, g:g + 1],
                                        inv_sb[:, g:g + 1],
                                        127.0).then_inc(sem_qs, 1)

        # ---------------- PE: conv matmuls
        nc.tensor.wait_ge(sem_x, 16 * SPC * CI_CH)
        nc.tensor.wait_ge(sem_wm, SPC * CI_CH)
        for ti, (s, co, t) in enumerate(tiles):
            if ti >= NPS:
                nc.tensor.wait_ge(sem_a1, ti - NPS + 1)
            for c in range(CI_CH):
                for kp in range(KK):
                    off = (kp // 3) * S + (kp % 3) + t * NTILE
                    lhsT = wm[s][c][:, kp * DIM + co * 128:
                                    kp * DIM + co * 128 + 128]
                    rhs = xp[s][c][:, off: off + NTILE]
                    mm = nc.tensor.matmul(
                        psum[ti % NPS][:], lhsT, rhs,
                        start=(c == 0 and kp == 0),
                        stop=(c == CI_CH - 1 and kp == KK - 1))
            mm.then_inc(sem_mm, 1)

        # ---------------- ACT: pass 1 (bias->bf16 obuf), pass 2 (quantize)
        # Software-pipelined per group: A1(g) ... A2(g-1) so A1 keeps
        # draining PSUM while A2 waits on the group's DVE qscale.
        nc.scalar.wait_ge(sem_small, 32)

        def a1_group(g):
            s, co = groups[g]
            for t in range(NT):
                ti = g * NT + t
                nc.scalar.wait_ge(sem_mm, ti + 1)
                nc.scalar.activation(
                    obuf_tile(g, t), psum[ti % NPS][:], Ident,
                    bias=bmix_sb[:, co * SPC + s: co * SPC + s + 1],
                ).then_inc(sem_a1, 1)

        def a2_group(g):
            nc.scalar.wait_ge(sem_qs, g + 1)
            for t in range(NT):
                ti = g * NT + t
                if ti >= NOUT:
                    nc.scalar.wait_ge(sem_outdma, 16 * (ti - NOUT + 1))
                nc.scalar.activation(
                    oti[ti % NOUT][:], obuf_tile(g, t), Copy,
                    scale=qs_sb[:, g:g + 1],
                ).then_inc(sem_a2, 1)

        for g in range(NG):
            a1_group(g)
            if g > 0:
                a2_group(g - 1)
        a2_group(NG - 1)

        # ---------------- SYNC: output DMAs
        for ti, (s, co, t) in enumerate(tiles):
            nc.sync.wait_ge(sem_a2, ti + 1)
            src = oti[ti % NOUT][:].rearrange("p (r u) -> p r u", u=S)[:, :, 0:W]
            nc.sync.dma_start(
                y[s, co * 128:(co + 1) * 128,
                  t * ROWS_PER_T:(t + 1) * ROWS_PER_T, :], src,
            ).then_inc(sem_outdma, 16)
        nc.sync.wait_ge(sem_qs, NG)
        nc.sync.dma_start(yscale[:, :], amax_sb[:]).then_inc(sem_outdma, 16)
        nc.sync.wait_ge(sem_outdma, 16 * (len(tiles) + 1))
    return nc


_STATE = None


def _get_state():
    global _STATE
    if _STATE is not None:
        return _STATE
    import jax
    import jax.numpy as jnp
    import ml_dtypes
    from jax.sharding import Mesh, PartitionSpec as P, NamedSharding
    from jax.experimental.shard_map import shard_map
    bass, tile, mybir = _imports()
    from concourse.bass2jax import (
        install_neuronx_cc_hook, _bass_exec_p, partition_id_tensor)

    install_neuronx_cc_hook()
    nc = build_bass_raw()

    partition_name = (nc.partition_id_tensor.name
                      if nc.partition_id_tensor else None)
    in_names, out_names, out_avals = [], [], []
    for alloc in nc.m.functions[0].allocations:
        if not isinstance(alloc, mybir.MemoryLocationSet):
            continue
        name = alloc.memorylocations[0].name
        if alloc.kind == "ExternalInput":
            if name != partition_name:
                in_names.append(name)
        elif alloc.kind == "ExternalOutput":
            out_names.append(name)
            out_avals.append(jax.core.ShapedArray(
                tuple(alloc.tensor_shape), mybir.dt.np(alloc.dtype)))
    n_params = len(in_names)
    in_names_all = in_names + out_names + (
        [partition_name] if partition_name else [])

    def _body(*args):
        operands = list(args)
        if partition_name is not None:
            operands.append(partition_id_tensor())
        outs = _bass_exec_p.bind(
            *operands, out_avals=tuple(out_avals),
            in_names=tuple(in_names_all), out_names=tuple(out_names),
            lowering_input_output_aliases=(),
            sim_require_finite=True, sim_require_nnan=True, nc=nc)
        return tuple(outs)

    devices = jax.devices()[:NCORES]
    mesh = Mesh(np.asarray(devices), ("core",))
    shard = NamedSharding(mesh, P("core"))
    n_ops = n_params + len(out_names)

    # The neuron compile cache keys on the HLO module (name/shapes) and
    # does NOT see the BIR embedded in the custom-call backend_config, so
    # two different Bass programs with identical operand shapes collide.
    # Name the jitted callable after the BIR content hash to disambiguate.
    import hashlib
    bh = hashlib.sha256(nc.to_json_bytes()).hexdigest()[:12]
    _sm_body = shard_map(_body, mesh=mesh, in_specs=(P("core"),) * n_ops,
                         out_specs=(P("core"),) * len(out_names),
                         check_rep=False)

    def _named_body(*args):
        return _sm_body(*args)
    _named_body.__name__ = f"bass_{bh}"
    _named_body.__qualname__ = _named_body.__name__
    sharded = jax.jit(_named_body, keep_unused=True)

    # Stage-1: all-gather the sharded native bank and transpose it into
    # matmul layout [n, ci, (kk co)] on device.
    def _st1(b):  # per-core (1, DIM*DIM*KK) bf16
        g = jax.lax.all_gather(b, "core", axis=0, tiled=True)  # (NK, ...)
        t = g.reshape(NK, DIM, DIM, KK).transpose(0, 2, 3, 1)  # n,ci,kk,co
        return t.reshape(NK, CI_CH, 128, KK * DIM)
    _sm_st1 = shard_map(_st1, mesh=mesh, in_specs=(P("core"),),
                        out_specs=P("core"), check_rep=False)

    def _named_st1(b):
        return _sm_st1(b)
    _named_st1.__name__ = "bank_gather_v1"
    _named_st1.__qualname__ = _named_st1.__name__
    st1 = jax.jit(_named_st1)

    # Persistent device-resident dummies for the output operands: the NEFF
    # binds outputs to fresh result buffers (the kernel writes every
    # element), so the operands' contents are never read. No upload.
    mkdummy = jax.jit(
        lambda: (jnp.zeros((B, DIM, H, W), jnp.int8),
                 jnp.zeros((NCORES * 128, NG), jnp.float32)),
        out_shardings=(shard, shard))
    ydummy, ysdummy = mkdummy()
    jax.block_until_ready((ydummy, ysdummy))

    _STATE = dict(jax=jax, ml_dtypes=ml_dtypes, nc=nc, sharded=sharded,
                  st1=st1, shard=shard, ydummy=ydummy, ysdummy=ysdummy)

    # Fully warm the pipeline (compiles all jits, primes transfer paths)
    # so the caller's steady-state calls see no lazy one-time costs. Two
    # passes: the axon staging layer can misdeliver the FIRST execution of
    # a freshly staged executable, so the first warmup absorbs that and
    # the second verifies steady state.
    rng = np.random.default_rng(0)
    wargs = (rng.standard_normal((B, DIM, H, W), dtype=np.float32),
             rng.random((B, NK), dtype=np.float32),
             rng.standard_normal((NK, DIM, DIM, KS, KS), dtype=np.float32),
             rng.standard_normal((NK, DIM), dtype=np.float32))
    kernel(*wargs)
    kernel(*wargs)
    return _STATE


def kernel(x, attention, weight, bias):
    st = _get_state()
    jax, ml_dtypes = st["jax"], st["ml_dtypes"]
    bf16 = ml_dtypes.bfloat16

    # x -> bf16 and start its upload first (biggest input); the rest of
    # the host prep overlaps with the transfer.
    x = np.asarray(x)
    xd = jax.device_put(x.astype(bf16), st["shard"])

    # Native-layout bank, bf16, sharded one kernel per core (9.4 MB total
    # on the wire); gathered + transposed on device by st1.
    weight = np.asarray(weight, dtype=np.float32)
    wbd = jax.device_put(weight.reshape(NK, -1).astype(bf16), st["shard"])
    gT = st["st1"](wbd)

    attention = np.asarray(attention, dtype=np.float32)
    attb = np.ascontiguousarray(np.broadcast_to(
        attention.reshape(NCORES, 1, SPC * NK),
        (NCORES, 128, SPC * NK))).reshape(NCORES * 128, SPC * NK)
    ad = jax.device_put(attb, st["shard"])

    bm = attention @ np.asarray(bias, dtype=np.float32)
    bmixT = np.ascontiguousarray(
        bm.reshape(NCORES, SPC, CO_CH, 128).transpose(0, 3, 2, 1)).reshape(
        NCORES * 128, CO_CH * SPC)
    bd = jax.device_put(bmixT, st["shard"])

    yarr, ysarr = st["sharded"](xd, gT, ad, bd, st["ydummy"], st["ysdummy"])

    # Dequantize: yscale column g = s_loc*CO_CH+co (s-major group order),
    # so scale[core*SPC+s_loc, co*128+p] = amax[core*128+p, g]/127.
    amax = np.asarray(ysarr).reshape(NCORES, 128, SPC, CO_CH)
    scale = (amax.transpose(0, 2, 3, 1) / np.float32(127.0)).reshape(
        B, DIM, 1, 1)
    out = np.asarray(yarr).astype(np.float32)
    out *= scale
    return out


# revision 16
# speedup vs baseline: 1.0492x; 1.0492x over previous
"""Trainium2 Bass kernel: per-sample dynamic conv (KernelAggregation).

Problem: out[b] = conv2d(x[b], sum_n att[b,n]*W[n], pad=1) + (att @ bias)[b]
  x: (16, 256, 56, 56) f32, att: (16, 8), W: (8, 256, 256, 3, 3), bias: (8, 256)

Sharding: data-parallel over batch, 2 samples per core across 8 cores.

The axon tunnel moves ~40-80 MB/s and the host has a single CPU, so wall
time is wire bytes + host byte-shuffling; device compute (~0.4 ms/core) is
free. Design:
  * x ships as bf16 (25.7 MB, not 51.4).
  * The weight bank ships ONCE, sharded (9.4 MB bf16, one kernel per
    core) in its native layout; a small stage-1 jax jit all-gathers and
    transposes it on device into matmul layout. No host transpose, no
    host mixing sgemm, no 151 MB replication.
  * The Bass kernel mixes per-sample conv weights on DVE via
    scalar_tensor_tensor FMA (acc = att[s,n]*bank[n] + acc), then runs
    the conv as 9 shifted bf16 matmuls accumulated in PSUM.
  * y returns as int8 (12.9 MB) with device-computed per-(sample,channel)
    abs-max scales (exact dynamic quantization: DVE abs-max reduction ->
    reciprocal -> ACT rescale+cast); the host dequantizes. Measured
    rel-err vs the fp32 reference stays ~1e-2, well inside 2e-2.
  * Everything dispatches async; the only blocking point is the final y
    fetch. The shard_map jit is built once at module scope; warm calls
    pay only transfers + one dispatch chain.
  * The donated-zero output upload of run_bass_kernel_spmd's axon path is
    replaced by persistent on-device dummy operands (the kernel writes
    every output element, so zero-init is unnecessary).
"""

import numpy as np
from contextlib import ExitStack

B, DIM, H, W = 16, 256, 56, 56
NK, KS = 8, 3
NCORES = 8
SPC = B // NCORES          # samples per core
S = W + 2                  # padded row stride (58)
NPAD = S * S               # 3364
XP_LEN = NPAD + 4          # slack so shifted reads stay in-bounds (even)
ROWS_PER_T = 8
NT = H // ROWS_PER_T       # 7 spatial tiles
NTILE = ROWS_PER_T * S     # 464 (matmul moving dim)
CI_CH = DIM // 128         # 2
CO_CH = DIM // 128         # 2
KK = KS * KS               # 9
NG = SPC * CO_CH           # quantization groups per core (4)

NPS = 4     # PSUM tiles
NOUT = 4    # output staging buffers


def _imports():
    try:
        import concourse.bass as bass  # noqa: F401
    except ImportError:
        import sys
        for p in ("/opt/trn_rl_repo",):
            if p not in sys.path:
                sys.path.insert(0, p)
    import concourse.bass as bass
    import concourse.tile as tile
    from concourse import mybir
    return bass, tile, mybir


def build_bass_raw():
    bass, tile, mybir = _imports()
    dt = mybir.dt
    nc = bass.Bass()

    x = nc.dram_tensor("x", [SPC, DIM, H, W], dt.bfloat16, kind="ExternalInput")
    # Device-gathered+transposed bank from the stage-1 jit: [n, ci, kk*co].
    gbank = nc.dram_tensor("gbank", [NK, CI_CH, 128, KK * DIM], dt.bfloat16,
                           kind="ExternalInput")
    attb = nc.dram_tensor("attb", [128, SPC * NK], dt.float32,
                          kind="ExternalInput")
    bmixT = nc.dram_tensor("bmixT", [128, CO_CH * SPC], dt.float32,
                           kind="ExternalInput")
    y = nc.dram_tensor("y", [SPC, DIM, H, W], dt.int8, kind="ExternalOutput")
    yscale = nc.dram_tensor("yscale", [128, NG], dt.float32,
                            kind="ExternalOutput")

    ctx = ExitStack()
    with ctx:
        sbm = lambda shape, name: ctx.enter_context(
            nc.sbuf_tensor(name, shape, dt.bfloat16))
        sbf = lambda shape, name: ctx.enter_context(
            nc.sbuf_tensor(name, shape, dt.float32))
        att_sb = sbf([128, SPC * NK], "att_sb")
        bmix_sb = sbf([128, CO_CH * SPC], "bmix_sb")
        xp = [[sbm([128, XP_LEN], f"xp{s}_{c}") for c in range(CI_CH)]
              for s in range(SPC)]
        bank = [[sbm([128, KK * DIM], f"bk{n}_{c}") for c in range(CI_CH)]
                for n in range(NK)]
        acc = [[sbf([128, KK * DIM], f"acc{s}_{c}") for c in range(CI_CH)]
               for s in range(SPC)]
        wm = [[sbm([128, KK * DIM], f"wm{s}_{c}") for c in range(CI_CH)]
              for s in range(SPC)]
        # Pre-quant y tiles, bf16, reusing the (consumed) acc region:
        # group g=(s,co) holds NT*NTILE=3248 of acc's 4608-bf16 capacity.
        obuf = [acc[g // CO_CH][g % CO_CH][:].bitcast(dt.bfloat16)
                for g in range(NG)]
        oti = [ctx.enter_context(nc.sbuf_tensor(f"oti{i}", [128, NTILE],
                                                dt.int8))
               for i in range(NOUT)]
        amax_sb = sbf([128, NG], "amax_sb")
        qs_sb = sbf([128, NG], "qs_sb")
        inv_sb = sbf([128, NG], "inv_sb")
        red0 = sbm([128, NT * NTILE], "red0")
        red1 = sbm([128, (NT * NTILE + 1) // 2], "red1")
        psum = [ctx.enter_context(nc.psum_tensor(f"ps{i}", [128, NTILE],
                                                 dt.float32))
                for i in range(NPS)]

        sem = lambda name: ctx.enter_context(nc.semaphore(name))
        sem_small = sem("sem_small")   # att+bmix DMA done (2x16)
        sem_ms = sem("sem_ms")         # DVE memsets done (1 each, 4)
        sem_x = sem("sem_x")           # x interior DMA done (4x16)
        sem_bank = sem("sem_bank")     # bank DMA (n,c) done at 16*(2n+c+1)
        sem_wm = sem("sem_wm")         # mixed weights (s,c) ready (4)
        sem_mm = sem("sem_mm")         # PE per-out-tile group done (28)
        sem_a1 = sem("sem_a1")         # ACT psum->obuf tiles (28)
        sem_qs = sem("sem_qs")         # DVE per-group qscale ready (4)
        sem_tree = sem("sem_tree")     # DVE reduce-tree level serializer
        sem_a2 = sem("sem_a2")         # ACT quantized tiles (28)
        sem_outdma = sem("sem_outdma")  # out DMA done (16 each, 28+1)

        Ident = mybir.ActivationFunctionType.Identity
        Copy = mybir.ActivationFunctionType.Copy
        Alu = mybir.AluOpType

        groups = [(s, co) for s in range(SPC) for co in range(CO_CH)]
        tiles = [(s, co, t) for (s, co) in groups for t in range(NT)]

        def obuf_tile(g, t):
            return obuf[g][:, t * NTILE:(t + 1) * NTILE]

        # ---------------- GPSIMD: all input DMAs
        nc.gpsimd.dma_start(att_sb[:], attb[:, :]).then_inc(sem_small, 16)
        nc.gpsimd.dma_start(bmix_sb[:], bmixT[:, :]).then_inc(sem_small, 16)
        for n in range(NK):
            for c in range(CI_CH):
                nc.gpsimd.dma_start(bank[n][c][:],
                                    gbank[n, c, :, :]).then_inc(sem_bank, 16)
        for i, (s, c) in enumerate([(s, c) for s in range(SPC)
                                    for c in range(CI_CH)]):
            nc.gpsimd.wait_ge(sem_ms, i + 1)
            interior = xp[s][c][:, :NPAD].rearrange(
                "p (r u) -> p r u", u=S)[:, 1:1 + H, 1:1 + W]
            nc.gpsimd.dma_start(
                interior, x[s, c * 128:(c + 1) * 128, :, :]).then_inc(sem_x, 16)

        # ---------------- DVE: memsets; weight mixing; y quant scales
        for s in range(SPC):
            for c in range(CI_CH):
                nc.vector.memset(xp[s][c][:].bitcast(dt.float32),
                                 0.0).then_inc(sem_ms, 1)
        nc.vector.wait_ge(sem_small, 16)   # att_sb loaded
        for n in range(NK):
            for c in range(CI_CH):
                nc.vector.wait_ge(sem_bank, 16 * (2 * n + c + 1))
                for s in range(SPC):
                    a = att_sb[:, s * NK + n: s * NK + n + 1]
                    if n == 0:
                        nc.vector.tensor_scalar_mul(
                            acc[s][c][:], bank[n][c][:], a)
                    else:
                        nc.vector.scalar_tensor_tensor(
                            acc[s][c][:], bank[n][c][:], a, acc[s][c][:],
                            Alu.mult, Alu.add)
        for s in range(SPC):
            for c in range(CI_CH):
                nc.vector.tensor_copy(wm[s][c][:],
                                      acc[s][c][:]).then_inc(sem_wm, 1)
        # Per-group abs-max over the 7 bf16 tiles: |x| = (x*-1) max x, then
        # a pairwise halving max tree (odd sizes overlap the middle column,
        # harmless for max). Dependent back-to-back DVE ops with partial
        # overlap misexecute on this toolchain, so every level is
        # serialized with a self-semaphore. tensor_reduce/abs_max
        # miscompile here, hence only mult/max STT ops.
        jt = 0
        for g in range(NG):
            ga = amax_sb[:, g:g + 1]
            nc.vector.wait_ge(sem_a1, (g + 1) * NT)
            n = NT * NTILE
            nc.vector.scalar_tensor_tensor(
                red0[:, :n], obuf[g][:, :n], -1.0, obuf[g][:, :n],
                Alu.mult, Alu.max).then_inc(sem_tree, 1)
            jt += 1
            cur, other = red0, red1
            while n > 1:
                h = (n + 1) // 2
                nc.vector.wait_ge(sem_tree, jt)
                nc.vector.scalar_tensor_tensor(
                    other[:, :h], cur[:, :h], 0.0, cur[:, n - h:n],
                    Alu.max, Alu.max).then_inc(sem_tree, 1)
                jt += 1
                cur, other = other, cur
                n = h
            nc.vector.wait_ge(sem_tree, jt)
            nc.vector.tensor_scalar_max(ga, cur[:, 0:1],
                                        1e-30).then_inc(sem_tree, 1)
            jt += 1
            nc.vector.wait_ge(sem_tree, jt)
            nc.vector.reciprocal(inv_sb[:, g:g + 1], ga).then_inc(sem_tree, 1)
            jt += 1
            nc.vector.wait_ge(sem_tree, jt)
            nc.vector.tensor_scalar_mul(qs_sb[:, g:g + 1],
                                        inv_sb[:, g:g + 1],
                                        127.0).then_inc(sem_qs, 1)

        # ---------------- PE: conv matmuls
        nc.tensor.wait_ge(sem_x, 16 * SPC * CI_CH)
        nc.tensor.wait_ge(sem_wm, SPC * CI_CH)
        for ti, (s, co, t) in enumerate(tiles):
            if ti >= NPS:
                nc.tensor.wait_ge(sem_a1, ti - NPS + 1)
            for c in range(CI_CH):
                for kp in range(KK):
                    off = (kp // 3) * S + (kp % 3) + t * NTILE
                    lhsT = wm[s][c][:, kp * DIM + co * 128:
                                    kp * DIM + co * 128 + 128]
                    rhs = xp[s][c][:, off: off + NTILE]
                    mm = nc.tensor.matmul(
                        psum[ti % NPS][:], lhsT, rhs,
                        start=(c == 0 and kp == 0),
                        stop=(c == CI_CH - 1 and kp == KK - 1))
            mm.then_inc(sem_mm, 1)

        # ---------------- ACT: pass 1 (bias->bf16 obuf), pass 2 (quantize)
        # Software-pipelined per group: A1(g) ... A2(g-1) so A1 keeps
        # draining PSUM while A2 waits on the group's DVE qscale.
        nc.scalar.wait_ge(sem_small, 32)

        def a1_group(g):
            s, co = groups[g]
            for t in range(NT):
                ti = g * NT + t
                nc.scalar.wait_ge(sem_mm, ti + 1)
                nc.scalar.activation(
                    obuf_tile(g, t), psum[ti % NPS][:], Ident,
                    bias=bmix_sb[:, co * SPC + s: co * SPC + s + 1],
                ).then_inc(sem_a1, 1)

        def a2_group(g):
            nc.scalar.wait_ge(sem_qs, g + 1)
            for t in range(NT):
                ti = g * NT + t
                if ti >= NOUT:
                    nc.scalar.wait_ge(sem_outdma, 16 * (ti - NOUT + 1))
                nc.scalar.activation(
                    oti[ti % NOUT][:], obuf_tile(g, t), Copy,
                    scale=qs_sb[:, g:g + 1],
                ).then_inc(sem_a2, 1)

        for g in range(NG):
            a1_group(g)
            if g > 0:
                a2_group(g - 1)
        a2_group(NG - 1)

        # ---------------- SYNC: output DMAs
        for ti, (s, co, t) in enumerate(tiles):
            nc.sync.wait_ge(sem_a2, ti + 1)
            src = oti[ti % NOUT][:].rearrange("p (r u) -> p r u", u=S)[:, :, 0:W]
            nc.sync.dma_start(
                y[s, co * 128:(co + 1) * 128,
                  t * ROWS_PER_T:(t + 1) * ROWS_PER_T, :], src,
            ).then_inc(sem_outdma, 16)
        nc.sync.wait_ge(sem_qs, NG)
        nc.sync.dma_start(yscale[:, :], amax_sb[:]).then_inc(sem_outdma, 16)
        nc.sync.wait_ge(sem_outdma, 16 * (len(tiles) + 1))
    return nc


_STATE = None


def _get_state():
    global _STATE
    if _STATE is not None:
        return _STATE
    import jax
    import jax.numpy as jnp
    import ml_dtypes
    from jax.sharding import Mesh, PartitionSpec as P, NamedSharding
    from jax.experimental.shard_map import shard_map
    bass, tile, mybir = _imports()
    from concourse.bass2jax import (
        install_neuronx_cc_hook, _bass_exec_p, partition_id_tensor)

    install_neuronx_cc_hook()
    nc = build_bass_raw()

    partition_name = (nc.partition_id_tensor.name
                      if nc.partition_id_tensor else None)
    in_names, out_names, out_avals = [], [], []
    for alloc in nc.m.functions[0].allocations:
        if not isinstance(alloc, mybir.MemoryLocationSet):
            continue
        name = alloc.memorylocations[0].name
        if alloc.kind == "ExternalInput":
            if name != partition_name:
                in_names.append(name)
        elif alloc.kind == "ExternalOutput":
            out_names.append(name)
            out_avals.append(jax.core.ShapedArray(
                tuple(alloc.tensor_shape), mybir.dt.np(alloc.dtype)))
    n_params = len(in_names)
    in_names_all = in_names + out_names + (
        [partition_name] if partition_name else [])

    def _body(*args):
        operands = list(args)
        if partition_name is not None:
            operands.append(partition_id_tensor())
        outs = _bass_exec_p.bind(
            *operands, out_avals=tuple(out_avals),
            in_names=tuple(in_names_all), out_names=tuple(out_names),
            lowering_input_output_aliases=(),
            sim_require_finite=True, sim_require_nnan=True, nc=nc)
        return tuple(outs)

    devices = jax.devices()[:NCORES]
    mesh = Mesh(np.asarray(devices), ("core",))
    shard = NamedSharding(mesh, P("core"))
    n_ops = n_params + len(out_names)

    # The neuron compile cache keys on the HLO module (name/shapes) and
    # does NOT see the BIR embedded in the custom-call backend_config, so
    # two different Bass programs with identical operand shapes collide.
    # Name the jitted callable after the BIR content hash to disambiguate.
    import hashlib
    bh = hashlib.sha256(nc.to_json_bytes()).hexdigest()[:12]
    _sm_body = shard_map(_body, mesh=mesh, in_specs=(P("core"),) * n_ops,
                         out_specs=(P("core"),) * len(out_names),
                         check_rep=False)

    def _named_body(*args):
        return _sm_body(*args)
    _named_body.__name__ = f"bass_{bh}"
    _named_body.__qualname__ = _named_body.__name__
    sharded = jax.jit(_named_body, keep_unused=True)

    # Stage-1: all-gather the sharded native bank and transpose it into
    # matmul layout [n, ci, (kk co)] on device.
    def _st1(b):  # per-core (1, DIM*DIM*KK) bf16
        g = jax.lax.all_gather(b, "core", axis=0, tiled=True)  # (NK, ...)
        t = g.reshape(NK, DIM, DIM, KK).transpose(0, 2, 3, 1)  # n,ci,kk,co
        return t.reshape(NK, CI_CH, 128, KK * DIM)
    _sm_st1 = shard_map(_st1, mesh=mesh, in_specs=(P("core"),),
                        out_specs=P("core"), check_rep=False)

    def _named_st1(b):
        return _sm_st1(b)
    _named_st1.__name__ = "bank_gather_v1"
    _named_st1.__qualname__ = _named_st1.__name__
    st1 = jax.jit(_named_st1)

    # Persistent device-resident dummies for the output operands: the NEFF
    # binds outputs to fresh result buffers (the kernel writes every
    # element), so the operands' contents are never read. No upload.
    mkdummy = jax.jit(
        lambda: (jnp.zeros((B, DIM, H, W), jnp.int8),
                 jnp.zeros((NCORES * 128, NG), jnp.float32)),
        out_shardings=(shard, shard))
    ydummy, ysdummy = mkdummy()
    jax.block_until_ready((ydummy, ysdummy))

    _STATE = dict(jax=jax, ml_dtypes=ml_dtypes, nc=nc, sharded=sharded,
                  st1=st1, shard=shard, ydummy=ydummy, ysdummy=ysdummy)

    # Fully warm the pipeline (compiles all jits, primes transfer paths)
    # so the caller's steady-state calls see no lazy one-time costs. Two
    # passes: the axon staging layer can misdeliver the FIRST execution of
    # a freshly staged executable, so the first warmup absorbs that and
    # the second verifies steady state.
    rng = np.random.default_rng(0)
    wargs = (rng.standard_normal((B, DIM, H, W), dtype=np.float32),
             rng.random((B, NK), dtype=np.float32),
             rng.standard_normal((NK, DIM, DIM, KS, KS), dtype=np.float32),
             rng.standard_normal((NK, DIM), dtype=np.float32))
    kernel(*wargs)
    kernel(*wargs)
    return _STATE


def kernel(x, attention, weight, bias):
    st = _get_state()
    jax, ml_dtypes = st["jax"], st["ml_dtypes"]
    bf16 = ml_dtypes.bfloat16

    # x -> bf16 and start its upload first (biggest input); the rest of
    # the host prep overlaps with the transfer.
    x = np.asarray(x)
    xd = jax.device_put(x.astype(bf16), st["shard"])

    # Native-layout bank, bf16, sharded one kernel per core (9.4 MB total
    # on the wire); gathered + transposed on device by st1.
    weight = np.asarray(weight, dtype=np.float32)
    wbd = jax.device_put(weight.reshape(NK, -1).astype(bf16), st["shard"])
    gT = st["st1"](wbd)

    attention = np.asarray(attention, dtype=np.float32)
    attb = np.ascontiguousarray(np.broadcast_to(
        attention.reshape(NCORES, 1, SPC * NK),
        (NCORES, 128, SPC * NK))).reshape(NCORES * 128, SPC * NK)
    ad = jax.device_put(attb, st["shard"])

    bm = attention @ np.asarray(bias, dtype=np.float32)
    bmixT = np.ascontiguousarray(
        bm.reshape(NCORES, SPC, CO_CH, 128).transpose(0, 3, 2, 1)).reshape(
        NCORES * 128, CO_CH * SPC)
    bd = jax.device_put(bmixT, st["shard"])

    yarr, ysarr = st["sharded"](xd, gT, ad, bd, st["ydummy"], st["ysdummy"])

    # Start both D2H transfers; the tiny yscale rides along with y instead
    # of paying its own blocking round-trip.
    for arr in (ysarr, yarr):
        for sh in arr.addressable_shards:
            sh.data.copy_to_host_async()
    yi = np.asarray(yarr)

    # Dequantize: yscale column g = s_loc*CO_CH+co (s-major group order),
    # so scale[core*SPC+s_loc, co*128+p] = amax[core*128+p, g]/127.
    amax = np.asarray(ysarr).reshape(NCORES, 128, SPC, CO_CH)
    scale = (amax.transpose(0, 2, 3, 1) / np.float32(127.0)).reshape(
        B, DIM, 1, 1)
    out = np.empty((B, DIM, H, W), np.float32)
    np.multiply(yi, scale, out=out, casting="unsafe")
    return out


# revision 18
# speedup vs baseline: 1.7661x; 1.6832x over previous
"""Trainium2 Bass kernel: per-sample dynamic conv (KernelAggregation).

Problem: out[b] = conv2d(x[b], sum_n att[b,n]*W[n], pad=1) + (att @ bias)[b]
  x: (16, 256, 56, 56) f32, att: (16, 8), W: (8, 256, 256, 3, 3), bias: (8, 256)

Sharding: data-parallel over batch, 2 samples per core across 8 cores.

The axon tunnel moves ~40 MB/s each way and the host has a single CPU, so
wall time is wire bytes + host byte passes; device compute (~0.5 ms/core)
is free. Wire format is int8 everywhere, compute is bf16/fp32:
  * x ships as int8 (12.9 MB) with per-(sample,channel) abs-max scales;
    the scale folds into the mixed conv weights on device (w rows are
    multiplied by sx[ci]), so SBUF x stays exact +-127 integers in bf16.
  * The weight bank ships ONCE, sharded (4.7 MB int8, one kernel per
    core, per-(kernel,ci) scales); a stage-1 jax jit all-gathers and
    transposes it on device. The bank scale folds into the DVE mixing
    scalars (att[s,n]*tw[n,ci] per partition), accumulated in fp32.
  * The Bass kernel mixes per-sample conv weights on DVE via
    scalar_tensor_tensor FMA, then runs the conv as 9 shifted bf16
    matmuls accumulated in PSUM.
  * y returns as int8 (12.9 MB) with device-computed per-(sample,channel)
    abs-max scales (DVE max tree -> reciprocal -> ACT rescale+cast); the
    host dequantizes in one fused pass. Measured rel-err ~1.5e-2 < 2e-2.
  * Everything dispatches async; the only blocking point is the final y
    fetch. All jits are built once at module scope; warm calls pay only
    transfers + one dispatch chain.

Toolchain landmines encoded below: the neuron compile cache keys miss the
BIR inside the custom-call backend_config (so jit names embed a BIR
hash); the first execution of a freshly staged executable can misdeliver
(so warmup runs twice); dependent DVE ops with partially overlapping
operands misexecute unless serialized with a semaphore; tensor_reduce
and the abs_max ALU op miscompile outright.
"""

import numpy as np
from contextlib import ExitStack

B, DIM, H, W = 16, 256, 56, 56
NK, KS = 8, 3
NCORES = 8
SPC = B // NCORES          # samples per core
S = W + 2                  # padded row stride (58)
NPAD = S * S               # 3364
XP_LEN = NPAD + 4          # slack so shifted reads stay in-bounds (even)
ROWS_PER_T = 8
NT = H // ROWS_PER_T       # 7 spatial tiles
NTILE = ROWS_PER_T * S     # 464 (matmul moving dim)
CI_CH = DIM // 128         # 2
CO_CH = DIM // 128         # 2
KK = KS * KS               # 9
NG = SPC * CO_CH           # quantization groups per core (4)
HW = H * W                 # 3136

# scl column layout (all per-partition f32 vectors, [128, NSCL] per core)
SC_ATW = 0                          # 32 cols: att[s,n]*tw[n,ci], s*16+n*2+c
SC_SX = SC_ATW + SPC * NK * CI_CH   # 4 cols: sx[s, ci], s*2+c
SC_BIAS = SC_SX + SPC * CI_CH       # 4 cols: bias_mix[s, co], co*2+s
NSCL = SC_BIAS + CO_CH * SPC        # 40

NPS = 4     # PSUM tiles
NOUT = 4    # output staging buffers


def _imports():
    try:
        import concourse.bass as bass  # noqa: F401
    except ImportError:
        import sys
        for p in ("/opt/trn_rl_repo",):
            if p not in sys.path:
                sys.path.insert(0, p)
    import concourse.bass as bass
    import concourse.tile as tile
    from concourse import mybir
    return bass, tile, mybir


def build_bass_raw():
    bass, tile, mybir = _imports()
    dt = mybir.dt
    nc = bass.Bass()

    xq = nc.dram_tensor("xq", [SPC, DIM, H, W], dt.int8, kind="ExternalInput")
    # Device-gathered+transposed int8 bank from stage-1: [n, ci, kk*co].
    gbank = nc.dram_tensor("gbank", [NK, CI_CH, 128, KK * DIM], dt.int8,
                           kind="ExternalInput")
    scl = nc.dram_tensor("scl", [128, NSCL], dt.float32, kind="ExternalInput")
    y = nc.dram_tensor("y", [SPC, DIM, H, W], dt.int8, kind="ExternalOutput")
    yscale = nc.dram_tensor("yscale", [128, NG], dt.float32,
                            kind="ExternalOutput")

    ctx = ExitStack()
    with ctx:
        sbm = lambda shape, name: ctx.enter_context(
            nc.sbuf_tensor(name, shape, dt.bfloat16))
        sbf = lambda shape, name: ctx.enter_context(
            nc.sbuf_tensor(name, shape, dt.float32))
        sb8 = lambda shape, name: ctx.enter_context(
            nc.sbuf_tensor(name, shape, dt.int8))
        scl_sb = sbf([128, NSCL], "scl_sb")
        xp = [[sbm([128, XP_LEN], f"xp{s}_{c}") for c in range(CI_CH)]
              for s in range(SPC)]
        xq_sb = [[sb8([128, HW], f"xq{s}_{c}") for c in range(CI_CH)]
                 for s in range(SPC)]
        bank = [[sb8([128, KK * DIM], f"bk{n}_{c}") for c in range(CI_CH)]
                for n in range(NK)]
        acc = [[sbf([128, KK * DIM], f"acc{s}_{c}") for c in range(CI_CH)]
               for s in range(SPC)]
        wm = [[sbm([128, KK * DIM], f"wm{s}_{c}") for c in range(CI_CH)]
              for s in range(SPC)]
        # Pre-quant y tiles, bf16, reusing the (consumed) acc region.
        obuf = [acc[g // CO_CH][g % CO_CH][:].bitcast(dt.bfloat16)
                for g in range(NG)]
        oti = [sb8([128, NTILE], f"oti{i}") for i in range(NOUT)]
        amax_sb = sbf([128, NG], "amax_sb")
        qs_sb = sbf([128, NG], "qs_sb")
        inv_sb = sbf([128, NG], "inv_sb")
        red0 = sbm([128, NT * NTILE], "red0")
        red1 = sbm([128, (NT * NTILE + 1) // 2], "red1")
        psum = [ctx.enter_context(nc.psum_tensor(f"ps{i}", [128, NTILE],
                                                 dt.float32))
                for i in range(NPS)]

        sem = lambda name: ctx.enter_context(nc.semaphore(name))
        sem_small = sem("sem_small")   # scl DMA done (16)
        sem_ms = sem("sem_ms")         # DVE memsets done (1 each, 4)
        sem_xq = sem("sem_xq")         # xq DMA done (4x16)
        sem_xp = sem("sem_xp")         # DVE int8->bf16 image writes (4)
        sem_bank = sem("sem_bank")     # bank DMA (n,c) done at 16*(2n+c+1)
        sem_wm = sem("sem_wm")         # mixed weights (s,c) ready (4)
        sem_mm = sem("sem_mm")         # PE per-out-tile group done (28)
        sem_a1 = sem("sem_a1")         # ACT psum->obuf tiles (28)
        sem_qs = sem("sem_qs")         # DVE per-group qscale ready (4)
        sem_tree = sem("sem_tree")     # DVE reduce-tree serializer
        sem_a2 = sem("sem_a2")         # ACT quantized tiles (28)
        sem_outdma = sem("sem_outdma")  # out DMA done (16 each, 28+1)

        Ident = mybir.ActivationFunctionType.Identity
        Copy = mybir.ActivationFunctionType.Copy
        Alu = mybir.AluOpType

        groups = [(s, co) for s in range(SPC) for co in range(CO_CH)]
        tiles = [(s, co, t) for (s, co) in groups for t in range(NT)]

        def obuf_tile(g, t):
            return obuf[g][:, t * NTILE:(t + 1) * NTILE]

        # ---------------- GPSIMD: all input DMAs
        nc.gpsimd.dma_start(scl_sb[:], scl[:, :]).then_inc(sem_small, 16)
        for i, (s, c) in enumerate([(s, c) for s in range(SPC)
                                    for c in range(CI_CH)]):
            nc.gpsimd.dma_start(
                xq_sb[s][c][:],
                xq[s, c * 128:(c + 1) * 128, :, :]).then_inc(sem_xq, 16)
        for n in range(NK):
            for c in range(CI_CH):
                nc.gpsimd.dma_start(bank[n][c][:],
                                    gbank[n, c, :, :]).then_inc(sem_bank, 16)

        # ---------------- DVE: memsets; x dequant-to-image; weight mixing
        for s in range(SPC):
            for c in range(CI_CH):
                nc.vector.memset(xp[s][c][:].bitcast(dt.float32),
                                 0.0).then_inc(sem_ms, 1)
        for i, (s, c) in enumerate([(s, c) for s in range(SPC)
                                    for c in range(CI_CH)]):
            nc.vector.wait_ge(sem_xq, 16 * (i + 1))
            interior = xp[s][c][:, :NPAD].rearrange(
                "p (r u) -> p r u", u=S)[:, 1:1 + H, 1:1 + W]
            src = xq_sb[s][c][:].rearrange("p (r u) -> p r u", u=W)
            nc.vector.tensor_copy(interior, src).then_inc(sem_xp, 1)
        nc.vector.wait_ge(sem_small, 16)
        for n in range(NK):
            for c in range(CI_CH):
                nc.vector.wait_ge(sem_bank, 16 * (2 * n + c + 1))
                for s in range(SPC):
                    a = scl_sb[:, SC_ATW + s * NK * CI_CH + n * CI_CH + c:
                               SC_ATW + s * NK * CI_CH + n * CI_CH + c + 1]
                    if n == 0:
                        nc.vector.tensor_scalar_mul(
                            acc[s][c][:], bank[n][c][:], a)
                    else:
                        nc.vector.scalar_tensor_tensor(
                            acc[s][c][:], bank[n][c][:], a, acc[s][c][:],
                            Alu.mult, Alu.add)
        for s in range(SPC):
            for c in range(CI_CH):
                sx = scl_sb[:, SC_SX + s * CI_CH + c:
                            SC_SX + s * CI_CH + c + 1]
                nc.vector.tensor_scalar_mul(wm[s][c][:], acc[s][c][:],
                                            sx).then_inc(sem_wm, 1)
        # Per-group abs-max over the 7 bf16 tiles: |x| = (x*-1) max x, then
        # a pairwise halving max tree (odd sizes overlap the middle column,
        # harmless for max). Dependent back-to-back DVE ops with partial
        # overlap misexecute on this toolchain, so every level is
        # serialized with a self-semaphore. tensor_reduce/abs_max
        # miscompile here, hence only mult/max STT ops.
        jt = 0
        for g in range(NG):
            ga = amax_sb[:, g:g + 1]
            nc.vector.wait_ge(sem_a1, (g + 1) * NT)
            n = NT * NTILE
            nc.vector.scalar_tensor_tensor(
                red0[:, :n], obuf[g][:, :n], -1.0, obuf[g][:, :n],
                Alu.mult, Alu.max).then_inc(sem_tree, 1)
            jt += 1
            cur, other = red0, red1
            while n > 1:
                h = (n + 1) // 2
                nc.vector.wait_ge(sem_tree, jt)
                nc.vector.scalar_tensor_tensor(
                    other[:, :h], cur[:, :h], 0.0, cur[:, n - h:n],
                    Alu.max, Alu.max).then_inc(sem_tree, 1)
                jt += 1
                cur, other = other, cur
                n = h
            nc.vector.wait_ge(sem_tree, jt)
            nc.vector.tensor_scalar_max(ga, cur[:, 0:1],
                                        1e-30).then_inc(sem_tree, 1)
            jt += 1
            nc.vector.wait_ge(sem_tree, jt)
            nc.vector.reciprocal(inv_sb[:, g:g + 1], ga).then_inc(sem_tree, 1)
            jt += 1
            nc.vector.wait_ge(sem_tree, jt)
            nc.vector.tensor_scalar_mul(qs_sb[:, g:g + 1],
                                        inv_sb[:, g:g + 1],
                                        127.0).then_inc(sem_qs, 1)

        # ---------------- PE: conv matmuls
        nc.tensor.wait_ge(sem_xp, SPC * CI_CH)
        nc.tensor.wait_ge(sem_wm, SPC * CI_CH)
        for ti, (s, co, t) in enumerate(tiles):
            if ti >= NPS:
                nc.tensor.wait_ge(sem_a1, ti - NPS + 1)
            for c in range(CI_CH):
                for kp in range(KK):
                    off = (kp // 3) * S + (kp % 3) + t * NTILE
                    lhsT = wm[s][c][:, kp * DIM + co * 128:
                                    kp * DIM + co * 128 + 128]
                    rhs = xp[s][c][:, off: off + NTILE]
                    mm = nc.tensor.matmul(
                        psum[ti % NPS][:], lhsT, rhs,
                        start=(c == 0 and kp == 0),
                        stop=(c == CI_CH - 1 and kp == KK - 1))
            mm.then_inc(sem_mm, 1)

        # ---------------- ACT: pass 1 (bias->bf16 obuf), pass 2 (quantize)
        # Software-pipelined per group: A1(g) ... A2(g-1) so A1 keeps
        # draining PSUM while A2 waits on the group's DVE qscale.
        nc.scalar.wait_ge(sem_small, 16)

        def a1_group(g):
            s, co = groups[g]
            bias = scl_sb[:, SC_BIAS + co * SPC + s:
                          SC_BIAS + co * SPC + s + 1]
            for t in range(NT):
                ti = g * NT + t
                nc.scalar.wait_ge(sem_mm, ti + 1)
                nc.scalar.activation(
                    obuf_tile(g, t), psum[ti % NPS][:], Ident,
                    bias=bias).then_inc(sem_a1, 1)

        def a2_group(g):
            nc.scalar.wait_ge(sem_qs, g + 1)
            for t in range(NT):
                ti = g * NT + t
                if ti >= NOUT:
                    nc.scalar.wait_ge(sem_outdma, 16 * (ti - NOUT + 1))
                nc.scalar.activation(
                    oti[ti % NOUT][:], obuf_tile(g, t), Copy,
                    scale=qs_sb[:, g:g + 1],
                ).then_inc(sem_a2, 1)

        for g in range(NG):
            a1_group(g)
            if g > 0:
                a2_group(g - 1)
        a2_group(NG - 1)

        # ---------------- SYNC: output DMAs
        for ti, (s, co, t) in enumerate(tiles):
            nc.sync.wait_ge(sem_a2, ti + 1)
            src = oti[ti % NOUT][:].rearrange("p (r u) -> p r u", u=S)[:, :, 0:W]
            nc.sync.dma_start(
                y[s, co * 128:(co + 1) * 128,
                  t * ROWS_PER_T:(t + 1) * ROWS_PER_T, :], src,
            ).then_inc(sem_outdma, 16)
        nc.sync.wait_ge(sem_qs, NG)
        nc.sync.dma_start(yscale[:, :], amax_sb[:]).then_inc(sem_outdma, 16)
        nc.sync.wait_ge(sem_outdma, 16 * (len(tiles) + 1))
    return nc


_STATE = None


def _get_state():
    global _STATE
    if _STATE is not None:
        return _STATE
    import jax
    import jax.numpy as jnp
    import ml_dtypes
    from jax.sharding import Mesh, PartitionSpec as P, NamedSharding
    from jax.experimental.shard_map import shard_map
    bass, tile, mybir = _imports()
    from concourse.bass2jax import (
        install_neuronx_cc_hook, _bass_exec_p, partition_id_tensor)

    install_neuronx_cc_hook()
    nc = build_bass_raw()

    partition_name = (nc.partition_id_tensor.name
                      if nc.partition_id_tensor else None)
    in_names, out_names, out_avals = [], [], []
    for alloc in nc.m.functions[0].allocations:
        if not isinstance(alloc, mybir.MemoryLocationSet):
            continue
        name = alloc.memorylocations[0].name
        if alloc.kind == "ExternalInput":
            if name != partition_name:
                in_names.append(name)
        elif alloc.kind == "ExternalOutput":
            out_names.append(name)
            out_avals.append(jax.core.ShapedArray(
                tuple(alloc.tensor_shape), mybir.dt.np(alloc.dtype)))
    n_params = len(in_names)
    in_names_all = in_names + out_names + (
        [partition_name] if partition_name else [])

    def _body(*args):
        operands = list(args)
        if partition_name is not None:
            operands.append(partition_id_tensor())
        outs = _bass_exec_p.bind(
            *operands, out_avals=tuple(out_avals),
            in_names=tuple(in_names_all), out_names=tuple(out_names),
            lowering_input_output_aliases=(),
            sim_require_finite=True, sim_require_nnan=True, nc=nc)
        return tuple(outs)

    devices = jax.devices()[:NCORES]
    mesh = Mesh(np.asarray(devices), ("core",))
    shard = NamedSharding(mesh, P("core"))
    n_ops = n_params + len(out_names)

    # The neuron compile cache keys on the HLO module (name/shapes) and
    # does NOT see the BIR embedded in the custom-call backend_config, so
    # two different Bass programs with identical operand shapes collide.
    # Name the jitted callable after the BIR content hash to disambiguate.
    import hashlib
    bh = hashlib.sha256(nc.to_json_bytes()).hexdigest()[:12]
    _sm_body = shard_map(_body, mesh=mesh, in_specs=(P("core"),) * n_ops,
                         out_specs=(P("core"),) * len(out_names),
                         check_rep=False)

    def _named_body(*args):
        return _sm_body(*args)
    _named_body.__name__ = f"bass_{bh}"
    _named_body.__qualname__ = _named_body.__name__
    sharded = jax.jit(_named_body, keep_unused=True)

    # Stage-1: all-gather the sharded native int8 bank and transpose it
    # into matmul layout [n, ci, (kk co)] on device.
    def _st1(b):  # per-core (1, DIM*DIM*KK) int8
        g = jax.lax.all_gather(b, "core", axis=0, tiled=True)  # (NK, ...)
        t = g.reshape(NK, DIM, DIM, KK).transpose(0, 2, 3, 1)  # n,ci,kk,co
        return t.reshape(NK, CI_CH, 128, KK * DIM)
    _sm_st1 = shard_map(_st1, mesh=mesh, in_specs=(P("core"),),
                        out_specs=P("core"), check_rep=False)

    def _named_st1(b):
        return _sm_st1(b)
    _named_st1.__name__ = "bank_gather_i8_v1"
    _named_st1.__qualname__ = _named_st1.__name__
    st1 = jax.jit(_named_st1)

    # Persistent device-resident dummies for the output operands: the NEFF
    # binds outputs to fresh result buffers (the kernel writes every
    # element), so the operands' contents are never read. No upload.
    mkdummy = jax.jit(
        lambda: (jnp.zeros((B, DIM, H, W), jnp.int8),
                 jnp.zeros((NCORES * 128, NG), jnp.float32)),
        out_shardings=(shard, shard))
    ydummy, ysdummy = mkdummy()
    jax.block_until_ready((ydummy, ysdummy))

    # Preallocated host scratch (page-fault once here, not per call).
    scratch = {
        "f32": np.empty((B, DIM, H, W), np.float32),
        "i8": np.empty((B, DIM, H, W), np.int8),
        "out": np.empty((B, DIM, H, W), np.float32),
    }
    for a in scratch.values():
        a.fill(0)

    _STATE = dict(jax=jax, ml_dtypes=ml_dtypes, nc=nc, sharded=sharded,
                  st1=st1, shard=shard, ydummy=ydummy, ysdummy=ysdummy,
                  scratch=scratch)

    # Fully warm the pipeline (compiles all jits, primes transfer paths)
    # so the caller's steady-state calls see no lazy one-time costs. Two
    # passes: the axon staging layer can misdeliver the FIRST execution of
    # a freshly staged executable, so the first warmup absorbs that and
    # the second verifies steady state.
    rng = np.random.default_rng(0)
    wargs = (rng.standard_normal((B, DIM, H, W), dtype=np.float32),
             rng.random((B, NK), dtype=np.float32),
             rng.standard_normal((NK, DIM, DIM, KS, KS), dtype=np.float32),
             rng.standard_normal((NK, DIM), dtype=np.float32))
    kernel(*wargs)
    kernel(*wargs)
    return _STATE


def kernel(x, attention, weight, bias):
    st = _get_state()
    jax, ml_dtypes = st["jax"], st["ml_dtypes"]
    bf16 = ml_dtypes.bfloat16
    sc = st["scratch"]

    # Quantize x to int8 with per-(sample,channel) abs-max scales and
    # start its upload first; the rest of the host prep overlaps the wire.
    x = np.asarray(x)
    np.abs(x, out=sc["f32"])
    amax_x = np.maximum(sc["f32"].max(axis=(2, 3)), 1e-30)  # (B, DIM)
    sx = (amax_x / np.float32(127.0)).astype(np.float32)
    np.multiply(x, (np.float32(127.0) / amax_x)[:, :, None, None],
                out=sc["f32"])
    np.rint(sc["f32"], out=sc["f32"])
    xq = sc["f32"].astype(np.int8)
    xd = jax.device_put(xq, st["shard"])

    # Quantize the native-layout bank to int8 with per-(kernel,ci) scales;
    # gathered + transposed on device by st1.
    weight = np.asarray(weight, dtype=np.float32)
    aw = np.abs(weight).max(axis=(1, 3, 4))                 # (NK, DIM) ci
    tw = (np.maximum(aw, 1e-30) / np.float32(127.0)).astype(np.float32)
    bq = np.rint(weight * (np.float32(1.0) / tw)[:, None, :, None, None]
                 ).astype(np.int8)
    wbd = jax.device_put(bq.reshape(NK, -1), st["shard"])
    gT = st["st1"](wbd)

    # Per-core scale/bias vector bundle [128, NSCL].
    attention = np.asarray(attention, dtype=np.float32)
    bias = np.asarray(bias, dtype=np.float32)
    att3 = attention.reshape(NCORES, SPC, NK)
    tw_r = tw.reshape(NK, CI_CH, 128)
    scl = np.empty((NCORES, 128, NSCL), np.float32)
    # att_tw[k, p, s*16+n*2+c] = att[k,s,n] * tw[n,c,p]
    scl[:, :, SC_ATW:SC_SX] = np.einsum(
        "ksn,ncp->kpsnc", att3, tw_r).reshape(NCORES, 128, SPC * NK * CI_CH)
    scl[:, :, SC_SX:SC_BIAS] = sx.reshape(
        NCORES, SPC, CI_CH, 128).transpose(0, 3, 1, 2).reshape(
        NCORES, 128, SPC * CI_CH)
    bm = attention @ bias
    scl[:, :, SC_BIAS:] = bm.reshape(
        NCORES, SPC, CO_CH, 128).transpose(0, 3, 2, 1).reshape(
        NCORES, 128, CO_CH * SPC)
    sd = jax.device_put(scl.reshape(NCORES * 128, NSCL), st["shard"])

    yarr, ysarr = st["sharded"](xd, gT, sd, st["ydummy"], st["ysdummy"])

    # Start both D2H transfers; the tiny yscale rides along with y instead
    # of paying its own blocking round-trip.
    for arr in (ysarr, yarr):
        for sh in arr.addressable_shards:
            sh.data.copy_to_host_async()
    yi = np.asarray(yarr)

    # Dequantize: yscale column g = s_loc*CO_CH+co (s-major group order),
    # so scale[core*SPC+s_loc, co*128+p] = amax[core*128+p, g]/127.
    amax = np.asarray(ysarr).reshape(NCORES, 128, SPC, CO_CH)
    scale = (amax.transpose(0, 2, 3, 1) / np.float32(127.0)).reshape(
        B, DIM, 1, 1)
    out = np.empty((B, DIM, H, W), np.float32)
    np.multiply(yi, scale, out=out, casting="unsafe")
    return out
